# revision 5
# baseline (speedup 1.0000x reference)
"""Trainium2 Bass kernel for nn_BasicDecoder (cross-attention + MLP decoder block).

Sharding: 8 cores; core c owns batch b = c//2 and head-group g = c%2 (4 heads).
Because the reference reshapes the per-head attention output [B,H,Q,DH] with a
raw view to [B,Q,H*DH], output row-block [1024j, 1024(j+1)) of "summed" depends
ONLY on head j. Each core therefore computes a disjoint [4096, 512] slice of
the final output with zero cross-core communication.

On device everything is kept in transposed layout [feature(P), token(free)].
A host-side permutation of query tokens makes the reference's view-reshuffle
into contiguous tiles on device. LN gains/biases and the 1/sqrt(dh) scale are
folded into weights on the host; LN on device is pure (x-mu)*rsqrt(var+eps),
with stats computed by ones-matmuls (partition-dim reductions) and replicated
across partitions by a rank-1 matmul.
"""
import numpy as np
import ml_dtypes

import concourse.bass as bass
import concourse.tile as tile
from concourse import bacc, mybir
from concourse import bass_utils

F32 = mybir.dt.float32
F32R = mybir.dt.float32r
BF16 = mybir.dt.bfloat16
AF = mybir.ActivationFunctionType
ALU = mybir.AluOpType

B, Q, KV, D, H = 4, 8192, 1024, 1024, 8
DH = D // H            # 128
OUT_C = 512
HID = 4096
EPS = 1e-5
N_CORES = 8
HPC = H // 2           # heads per core = 4
ROWS = Q // 2          # output rows per core = 4096
SUB = 512
NSUB = Q // SUB        # 16 qtok subtiles
NSTRIP = 2             # strips of 4096 qtok

_CACHE = {}


def _query_perm():
    """perm[P] = original qtok index at permuted position P."""
    s = np.arange(NSTRIP)[:, None, None]
    u = np.arange(8)[None, :, None]
    rho = np.arange(SUB)[None, None, :]
    return (4096 * s + 8 * rho + u).reshape(-1)


def build(nrep=1, debug=False):
    nc = bacc.Bacc("TRN2", target_bir_lowering=False, debug=False,
                   enable_asserts=False)

    def din(name, shape, dt=F32R):
        return nc.dram_tensor(name, shape, dt, kind="ExternalInput").ap()

    qT = din("qT", [D, Q])
    zT = din("zT", [D, KV])
    wq = din("wq", [D, 512]); wk = din("wk", [D, 512]); wv = din("wv", [D, 512])
    wo = din("wo", [D, D])
    w1 = din("w1", [D, HID], BF16)
    w2 = din("w2", [HID, D], BF16)
    wf = din("wf", [D, OUT_C])
    bq = din("bq", [128, HPC], F32); bk = din("bk", [128, HPC], F32)
    bvb = din("bvb", [128, 512], F32)
    bo = din("bo", [128, 8], F32)
    b1 = din("b1", [128, 32], F32)
    b2 = din("b2", [128, 8], F32)
    bfp = din("bfp", [128, 4], F32)

    outT = nc.dram_tensor("outT", [OUT_C, ROWS], F32, kind="ExternalOutput").ap()

    if debug:
        d_kvn = nc.dram_tensor("d_kvn", [128, 8, KV], F32, kind="ExternalOutput").ap()
        d_K = nc.dram_tensor("d_K", [128, HPC, KV], F32, kind="ExternalOutput").ap()
        d_V = nc.dram_tensor("d_V", [128, 8, 512], F32, kind="ExternalOutput").ap()
        d_qn0 = nc.dram_tensor("d_qn0", [128, 8, SUB], F32, kind="ExternalOutput").ap()
        d_Q00 = nc.dram_tensor("d_Q00", [128, SUB], F32, kind="ExternalOutput").ap()
        d_O00 = nc.dram_tensor("d_O00", [128, 4096], F32, kind="ExternalOutput").ap()
        d_AO0 = nc.dram_tensor("d_AO0", [128, 8, SUB], F32, kind="ExternalOutput").ap()
        d_XN0 = nc.dram_tensor("d_XN0", [128, 8, SUB], F32, kind="ExternalOutput").ap()
        d_H0 = nc.dram_tensor("d_H0", [128, 32, SUB], F32, kind="ExternalOutput").ap()

    # [p, c, t] views of [D, N] dram tensors (D = 8 chunks x 128 partitions)
    qTv = qT.rearrange("(c p) t -> p c t", p=128)
    zTv = zT.rearrange("(c p) t -> p c t", p=128)
    wqv = wq.rearrange("(c p) n -> c p n", p=128)
    wkv = wk.rearrange("(c p) n -> c p n", p=128)
    wvv = wv.rearrange("(c p) n -> c p n", p=128)
    wov = wo.rearrange("(c p) n -> c p n", p=128)
    w1v = w1.rearrange("(c p) n -> p c n", p=128)
    w2v = w2.rearrange("(c p) n -> c p n", p=128)
    wfv = wf.rearrange("(c p) n -> c p n", p=128)

    with tile.TileContext(nc) as tc:
        with tc.tile_pool(name="outer", bufs=1) as outer, \
             tc.tile_pool(name="dstg", bufs=1, space="DRAM") as dstg:
            # ---- constants & biases ----
            ones_f = outer.tile([128, 128], F32)
            nc.gpsimd.memset(ones_f[:], 1.0)
            ones_col = outer.tile([128, 1], F32R)
            nc.vector.tensor_copy(ones_col[:], ones_f[:, 0:1])
            ones_row = outer.tile([1, 128], F32R)
            nc.vector.tensor_copy(ones_row[:], ones_f[0:1, :])
            ones128 = outer.tile([128, 128], F32R)
            nc.vector.tensor_copy(ones128[:], ones_f[:])
            epst = outer.tile([1, 1], F32)
            nc.gpsimd.memset(epst[:], EPS)
            bq_t = outer.tile([128, HPC], F32); nc.sync.dma_start(bq_t[:], bq)
            bk_t = outer.tile([128, HPC], F32); nc.sync.dma_start(bk_t[:], bk)
            bvb_t = outer.tile([128, 512], F32); nc.sync.dma_start(bvb_t[:], bvb)
            bo_t = outer.tile([128, 8], F32); nc.sync.dma_start(bo_t[:], bo)
            b1_t = outer.tile([128, 32], F32); nc.sync.dma_start(b1_t[:], b1)
            b2_t = outer.tile([128, 8], F32); nc.sync.dma_start(b2_t[:], b2)
            bf_t = outer.tile([128, 4], F32); nc.sync.dma_start(bf_t[:], bfp)

            ao_stg = dstg.tile([8, 128, 8, SUB], F32R)
            xn_stg = dstg.tile([8, 128, 8, SUB], BF16)
            o_stg = dstg.tile([NSTRIP, HPC, 128, 8, SUB], F32R)

            def ln_stats(ps1, ps2, sbp, rhs_chunks, n_feat, width, sq_maker):
                """Replicated LN stats: returns (mu_rep, r_rep) [128,width] f32."""
                s_ps = ps1.tile([1, width], F32, tag="stat_s")
                q_ps = ps1.tile([1, width], F32, tag="stat_q")
                nch = len(rhs_chunks)
                for c in range(nch):
                    nc.tensor.matmul(s_ps[:], ones_col[:], rhs_chunks[c],
                                     start=(c == 0), stop=(c == nch - 1))
                for c in range(nch):
                    nc.tensor.matmul(q_ps[:], ones_col[:], sq_maker(c),
                                     start=(c == 0), stop=(c == nch - 1))
                mu = sbp.tile([1, width], F32R, tag="mu")
                nc.vector.tensor_scalar_mul(mu[:], s_ps[:], 1.0 / n_feat)
                tmp = sbp.tile([1, width], F32, tag="ltmp")
                nc.vector.tensor_tensor(tmp[:], mu[:].bitcast(F32), s_ps[:],
                                        op=ALU.mult)
                v = sbp.tile([1, width], F32, tag="lvar")
                nc.vector.tensor_tensor(v[:], q_ps[:], tmp[:], op=ALU.subtract)
                lnv = sbp.tile([1, width], F32, tag="llnv")
                nc.scalar.activation(lnv[:], v[:], AF.Ln, bias=epst[:],
                                     scale=1.0 / n_feat)
                r = sbp.tile([1, width], F32R, tag="lr")
                nc.scalar.activation(r[:], lnv[:], AF.Exp, scale=-0.5)
                mur_ps = ps2.tile([128, width], F32, tag="repl")
                rr_ps = ps2.tile([128, width], F32, tag="repl")
                nc.tensor.matmul(mur_ps[:], ones_row[:], mu[:], start=True, stop=True)
                nc.tensor.matmul(rr_ps[:], ones_row[:], r[:], start=True, stop=True)
                mur = sbp.tile([128, width], F32, tag="murep")
                rr = sbp.tile([128, width], F32, tag="rrep")
                nc.vector.tensor_copy(mur[:], mur_ps[:])
                nc.vector.tensor_copy(rr[:], rr_ps[:])
                return mur, rr

            for _rep in range(nrep):
                with tc.tile_pool(name="pers", bufs=1) as pers:
                    wq_sb = pers.tile([128, 8, 512], F32R, tag="wq")
                    K_sb = pers.tile([128, HPC, KV], F32R, tag="K")
                    V_sb = pers.tile([128, 8, 512], F32R, tag="V")
                    for c in range(8):
                        nc.sync.dma_start(wq_sb[:, c], wqv[c])

                    # ================= KV stage =================
                    with tc.tile_pool(name="kvp1", bufs=1) as kvp1, \
                         tc.tile_pool(name="kvp2", bufs=2) as kvp2, \
                         tc.tile_pool(name="kvps1", bufs=1, space="PSUM") as kvps1, \
                         tc.tile_pool(name="kvps2", bufs=2, space="PSUM") as kvps2:
                        wk_sb = kvp1.tile([128, 8, 512], F32R, tag="wk")
                        wv_sb = kvp1.tile([128, 8, 512], F32R, tag="wv")
                        for c in range(8):
                            nc.sync.dma_start(wk_sb[:, c], wkv[c])
                            nc.sync.dma_start(wv_sb[:, c], wvv[c])
                        zt = kvp1.tile([128, 8, KV], F32R, tag="zt")
                        nc.sync.dma_start(zt[:], zTv)
                        for hf in range(2):
                            sl = slice(hf * 512, hf * 512 + 512)

                            def sqm(c, sl=sl):
                                t = kvp2.tile([128, 512], F32R, tag="zsq")
                                nc.scalar.activation(t[:], zt[:, c, sl], AF.Square)
                                return t[:]
                            mur, rr = ln_stats(kvps1, kvps2, kvp2,
                                               [zt[:, c, sl] for c in range(8)],
                                               D, 512, sqm)
                            for c in range(8):
                                t1 = kvp2.tile([128, 512], F32, tag="kt1")
                                nc.vector.tensor_tensor(
                                    t1[:], zt[:, c, sl].bitcast(F32), mur[:],
                                    op=ALU.subtract)
                                nc.vector.tensor_tensor(
                                    zt[:, c, sl], t1[:], rr[:], op=ALU.mult)
                        if debug:
                            for c in range(8):
                                nc.sync.dma_start(d_kvn[:, c], zt[:, c].bitcast(F32))
                        for h in range(HPC):
                            for hf in range(2):
                                sl = slice(hf * 512, hf * 512 + 512)
                                kps = kvps2.tile([128, 512], F32, tag="kwork")
                                for c in range(8):
                                    nc.tensor.matmul(
                                        kps[:], wk_sb[:, c, 128 * h:128 * h + 128],
                                        zt[:, c, sl], start=(c == 0), stop=(c == 7))
                                nc.vector.tensor_scalar_add(
                                    K_sb[:, h, sl], kps[:], bk_t[:, h:h + 1])
                        for kc in range(8):
                            vps = kvps2.tile([128, 512], F32, tag="vwork")
                            for c in range(8):
                                nc.tensor.matmul(
                                    vps[:], zt[:, c, 128 * kc:128 * kc + 128],
                                    wv_sb[:, c], start=(c == 0), stop=(c == 7))
                            nc.vector.tensor_tensor(
                                V_sb[:, kc], vps[:], bvb_t[:], op=ALU.add)
                        if debug:
                            for h in range(HPC):
                                nc.sync.dma_start(d_K[:, h], K_sb[:, h].bitcast(F32))
                            for kc in range(8):
                                nc.sync.dma_start(d_V[:, kc], V_sb[:, kc].bitcast(F32))

                    # ================= attention =================
                    with tc.tile_pool(name="attp", bufs=2) as attp, \
                         tc.tile_pool(name="attp3", bufs=3) as attp3, \
                         tc.tile_pool(name="aps1", bufs=1, space="PSUM") as aps1, \
                         tc.tile_pool(name="aps2", bufs=2, space="PSUM") as aps2:
                        for i in range(NSUB):
                            s, isub = divmod(i, 8)
                            qt = attp.tile([128, 8, SUB], F32R, tag="qt")
                            nc.sync.dma_start(qt[:],
                                              qTv[:, :, SUB * i:SUB * (i + 1)])

                            def sqm(c):
                                t = attp3.tile([128, SUB], F32R, tag="qsq")
                                nc.scalar.activation(t[:], qt[:, c], AF.Square)
                                return t[:]
                            mur, rr = ln_stats(aps1, aps2, attp,
                                               [qt[:, c] for c in range(8)],
                                               D, SUB, sqm)
                            for c in range(8):
                                t1 = attp.tile([128, SUB], F32, tag="qt1")
                                nc.vector.tensor_tensor(
                                    t1[:], qt[:, c].bitcast(F32), mur[:],
                                    op=ALU.subtract)
                                nc.vector.tensor_tensor(
                                    qt[:, c], t1[:], rr[:], op=ALU.mult)
                            if debug and i == 0:
                                for c in range(8):
                                    nc.sync.dma_start(d_qn0[:, c],
                                                      qt[:, c].bitcast(F32))
                            for h in range(HPC):
                                qps = aps2.tile([128, SUB], F32, tag="work")
                                for c in range(8):
                                    nc.tensor.matmul(
                                        qps[:], wq_sb[:, c, 128 * h:128 * h + 128],
                                        qt[:, c], start=(c == 0), stop=(c == 7))
                                Qh = attp.tile([128, SUB], F32R, tag="Qh")
                                nc.vector.tensor_scalar_add(Qh[:], qps[:],
                                                            bq_t[:, h:h + 1])
                                if debug and i == 0 and h == 0:
                                    nc.sync.dma_start(d_Q00, Qh[:].bitcast(F32))
                                ops = aps1.tile([128, SUB], F32, tag="ops")
                                dps = aps1.tile([128, SUB], F32, tag="dps")
                                for c in range(8):
                                    att = aps2.tile([128, SUB], F32, tag="work")
                                    nc.tensor.matmul(
                                        att[:], K_sb[:, h, 128 * c:128 * c + 128],
                                        Qh[:], start=True, stop=True)
                                    pc = attp3.tile([128, SUB], F32R, tag="pc")
                                    nc.scalar.activation(pc[:], att[:], AF.Exp)
                                    nc.tensor.matmul(
                                        ops[:], V_sb[:, c, 128 * h:128 * h + 128],
                                        pc[:], start=(c == 0), stop=(c == 7))
                                    nc.tensor.matmul(
                                        dps[:], ones128[:], pc[:],
                                        start=(c == 0), stop=(c == 7))
                                rec = attp.tile([128, SUB], F32, tag="rec")
                                nc.vector.reciprocal(rec[:], dps[:])
                                opc = attp3.tile([128, SUB], F32R, tag="opc")
                                nc.vector.tensor_tensor(
                                    opc[:], ops[:], rec[:], op=ALU.mult)
                                nc.sync.dma_start(o_stg[s, h, :, isub], opc[:])
                        if debug:
                            nc.sync.dma_start(
                                d_O00, o_stg[0, 0].rearrange("p c t -> p (c t)").bitcast(F32))

                    # ================= Wo + attn LN =================
                    with tc.tile_pool(name="wop1", bufs=1) as wop1, \
                         tc.tile_pool(name="wop2", bufs=2) as wop2, \
                         tc.tile_pool(name="wops1", bufs=1, space="PSUM") as wops1, \
                         tc.tile_pool(name="wops2", bufs=2, space="PSUM") as wops2:
                        wo_sb = wop1.tile([128, 8, D], F32R, tag="wo")
                        for c in range(8):
                            nc.sync.dma_start(wo_sb[:, c], wov[c])
                        for s in range(NSTRIP):
                            for h in range(HPC):
                                t = s * HPC + h
                                orhs = wop2.tile([128, 8, SUB], F32R, tag="orhs")
                                nc.sync.dma_start(orhs[:], o_stg[s, h])
                                AO = wop1.tile([128, 8, SUB], F32R, tag="AO")
                                for oc in range(8):
                                    aps = wops2.tile([128, SUB], F32, tag="aops")
                                    for u in range(8):
                                        nc.tensor.matmul(
                                            aps[:],
                                            wo_sb[:, u, 128 * oc:128 * oc + 128],
                                            orhs[:, u],
                                            start=(u == 0), stop=(u == 7))
                                    nc.vector.tensor_scalar_add(
                                        AO[:, oc], aps[:], bo_t[:, oc:oc + 1])
                                nc.sync.dma_start(ao_stg[t], AO[:])

                                def sqm(c):
                                    tq = wop2.tile([128, SUB], F32R, tag="aosq")
                                    nc.scalar.activation(tq[:], AO[:, c], AF.Square)
                                    return tq[:]
                                mur, rr = ln_stats(wops1, wops2, wop2,
                                                   [AO[:, c] for c in range(8)],
                                                   D, SUB, sqm)
                                xn = wop2.tile([128, 8, SUB], BF16, tag="xn")
                                for c in range(8):
                                    t1 = wop2.tile([128, SUB], F32, tag="wt1")
                                    nc.vector.tensor_tensor(
                                        t1[:], AO[:, c].bitcast(F32), mur[:],
                                        op=ALU.subtract)
                                    nc.vector.tensor_tensor(
                                        xn[:, c], t1[:], rr[:], op=ALU.mult)
                                nc.sync.dma_start(xn_stg[t], xn[:])
                                if debug and t == 0:
                                    for c in range(8):
                                        nc.sync.dma_start(d_AO0[:, c],
                                                          AO[:, c].bitcast(F32))
                                    nc.gpsimd.dma_start(d_XN0, xn[:])

                # ================= MLP + final projection =================
                with tc.tile_pool(name="p2h", bufs=1) as p2h, \
                     tc.tile_pool(name="p2b", bufs=2) as p2b, \
                     tc.tile_pool(name="p2ps", bufs=2, space="PSUM") as p2ps, \
                     tc.tile_pool(name="p2psx", bufs=1, space="PSUM") as p2psx:
                    w2_sb = p2h.tile([128, 32, D], BF16, tag="w2")
                    for c in range(32):
                        nc.sync.dma_start(w2_sb[:, c], w2v[c])
                    wf_sb = p2h.tile([128, 8, OUT_C], F32R, tag="wf")
                    for c in range(8):
                        nc.sync.dma_start(wf_sb[:, c], wfv[c])
                    for t in range(8):
                        s2, h2 = divmod(t, HPC)
                        rowoff = 1024 * h2 + 512 * s2
                        xn_t = p2h.tile([128, 8, SUB], BF16, tag="xnin")
                        nc.sync.dma_start(xn_t[:], xn_stg[t])
                        h_sb = p2h.tile([128, 32, SUB], BF16, tag="h")
                        for gb in range(4):
                            w1_blk = p2b.tile([128, 8, 1024], BF16, tag="w1blk")
                            nc.sync.dma_start(
                                w1_blk[:], w1v[:, :, 1024 * gb:1024 * (gb + 1)])
                            for gg in range(8):
                                G = gb * 8 + gg
                                hps = p2ps.tile([128, SUB], F32, tag="hps")
                                for c in range(8):
                                    nc.tensor.matmul(
                                        hps[:],
                                        w1_blk[:, c, 128 * gg:128 * gg + 128],
                                        xn_t[:, c], start=(c == 0), stop=(c == 7))
                                nc.scalar.activation(h_sb[:, G], hps[:], AF.Gelu,
                                                     bias=b1_t[:, G:G + 1])
                        if debug and t == 0:
                            nc.gpsimd.dma_start(d_H0, h_sb[:])
                        X = p2h.tile([128, 8, SUB], F32R, tag="X")
                        for half in range(2):
                            xps = p2psx.tile([128, 4, SUB], F32, tag="xps")
                            for G in range(32):
                                for oc4 in range(4):
                                    oc = 4 * half + oc4
                                    nc.tensor.matmul(
                                        xps[:, oc4],
                                        w2_sb[:, G, 128 * oc:128 * oc + 128],
                                        h_sb[:, G], start=(G == 0), stop=(G == 31))
                            ao_c = p2b.tile([128, 4, SUB], F32R, tag="aoc")
                            nc.sync.dma_start(
                                ao_c[:], ao_stg[t, :, 4 * half:4 * half + 4])
                            for oc4 in range(4):
                                oc = 4 * half + oc4
                                nc.vector.scalar_tensor_tensor(
                                    X[:, oc], xps[:, oc4], b2_t[:, oc:oc + 1],
                                    ao_c[:, oc4].bitcast(F32),
                                    op0=ALU.add, op1=ALU.add)
                        for of in range(4):
                            ofps = p2ps.tile([128, SUB], F32, tag="ofps")
                            for c in range(8):
                                nc.tensor.matmul(
                                    ofps[:], wf_sb[:, c, 128 * of:128 * of + 128],
                                    X[:, c], start=(c == 0), stop=(c == 7))
                            outt = p2b.tile([128, SUB], F32, tag="outt")
                            nc.vector.tensor_scalar_add(outt[:], ofps[:],
                                                        bf_t[:, of:of + 1])
                            nc.sync.dma_start(
                                outT[128 * of:128 * (of + 1),
                                     rowoff:rowoff + SUB], outt[:])
    nc.compile()
    return nc


def _prep_host(inputs):
    """Fold LN gains/biases + attention scale into weights; build per-core maps."""
    f64 = np.float64
    gq, bq_ln = inputs["ln_q_g"].astype(f64), inputs["ln_q_b"].astype(f64)
    gkv, bkv_ln = inputs["ln_kv_g"].astype(f64), inputs["ln_kv_b"].astype(f64)
    ga, ba_ln = inputs["ln_a_g"].astype(f64), inputs["ln_a_b"].astype(f64)
    Wq, Wk, Wv = (np.asarray(inputs[k], f64) for k in ("Wq", "Wk", "Wv"))
    Wo, W1, W2, Wf = (np.asarray(inputs[k], f64) for k in ("Wo", "W1", "W2", "Wf"))
    bq_, bk_, bv_ = (np.asarray(inputs[k], f64) for k in ("bq", "bk", "bv"))
    bo_, b1_, b2_, bf_ = (np.asarray(inputs[k], f64)
                          for k in ("bo", "b1", "b2", "bf"))

    sc = 1.0 / np.sqrt(DH)
    Wq_e = (gq[:, None] * Wq) * sc
    bq_e = (bq_ln @ Wq + bq_) * sc
    Wk_e = gkv[:, None] * Wk
    bk_e = bkv_ln @ Wk + bk_
    Wv_e = gkv[:, None] * Wv
    bv_e = bkv_ln @ Wv + bv_
    W1_e = ga[:, None] * W1
    b1_e = ba_ln @ W1 + b1_

    perm = _query_perm()
    f32 = np.float32
    query = np.asarray(inputs["query"], f32)
    z = np.asarray(inputs["z"], f32)
    maps = []
    shared = {
        "wo": np.ascontiguousarray(Wo.astype(f32)),
        "w1": np.ascontiguousarray(W1_e.astype(ml_dtypes.bfloat16)),
        "w2": np.ascontiguousarray(W2.astype(ml_dtypes.bfloat16)),
        "wf": np.ascontiguousarray(Wf.astype(f32)),
        "bo": np.ascontiguousarray(bo_.reshape(8, 128).T.astype(f32)),
        "b1": np.ascontiguousarray(b1_e.reshape(32, 128).T.astype(f32)),
        "b2": np.ascontiguousarray(b2_.reshape(8, 128).T.astype(f32)),
        "bfp": np.ascontiguousarray(bf_.reshape(4, 128).T.astype(f32)),
    }
    for core in range(N_CORES):
        b, g = divmod(core, 2)
        hs = slice(512 * g, 512 * (g + 1))
        m = dict(shared)
        m.update({
            "qT": np.ascontiguousarray(query[b][perm].T),
            "zT": np.ascontiguousarray(z[b].T),
            "wq": np.ascontiguousarray(Wq_e[:, hs].astype(f32)),
            "wk": np.ascontiguousarray(Wk_e[:, hs].astype(f32)),
            "wv": np.ascontiguousarray(Wv_e[:, hs].astype(f32)),
            "bq": np.ascontiguousarray(bq_e[hs].reshape(HPC, 128).T.astype(f32)),
            "bk": np.ascontiguousarray(bk_e[hs].reshape(HPC, 128).T.astype(f32)),
            "bvb": np.broadcast_to(bv_e[hs].astype(f32), (128, 512)).copy(),
        })
        maps.append(m)
    return maps


def kernel(**inputs):
    assert bool(np.all(inputs["query_mask"])), \
        "kernel specialization assumes all-ones query_mask"
    if "nc" not in _CACHE:
        _CACHE["nc"] = build()
    nc = _CACHE["nc"]
    maps = _prep_host(inputs)
    res = bass_utils.run_bass_kernel_spmd(nc, maps, core_ids=list(range(N_CORES)))
    out = np.empty((B, Q, OUT_C), dtype=np.float32)
    for core in range(N_CORES):
        b, g = divmod(core, 2)
        out[b, ROWS * g:ROWS * (g + 1), :] = res.results[core]["outT"].T
    return out
